# revision 23
# baseline (speedup 1.0000x reference)
"""Trainium2 Bass kernel for the C51-style categorical projection loss.

Math
----
The reference computes, per batch row i (direction d_i in {0,1}, scalar
skewness s):

    skewed_anchor[i] = anchor[i] @ P_{d_i}          (row-local scatter-add)
    loss = -mean_i( w_i * (skewed_anchor[i] . log(feature[i] + 1e-16)) )

P_d is a 51x51 projection matrix depending only on the scalar skew
(+s for d=0, -s for d=1).  Rearranging the triple sum, the whole loss
reduces to two 51x51 batch-contraction matrices:

    Mw [u,j] = sum_i w_i            * logf[i,u] * anchor[i,j]
    Ms [u,j] = sum_i w_i (1 - 2m_i) * logf[i,u] * anchor[i,j]
    loss = -( sum P0 . ((Mw+Ms)/2)^T + sum P1 . ((Mw-Ms)/2)^T ) / B

The host pre-scales anchor by the signed weight w*(1-2m) and ships it (and
feature) as bf16 — half the HBM traffic of f32.  On device, the unsigned
operand is recovered with a bitwise-and (abs on the bf16 sign bit), and the
contraction over batch rows maps straight onto the TensorEngine (batch rows
on the 128 partitions).  No scatter at all on device; the 51x51 results and
the tiny P matrices are combined on the host.

Sharding: pure data parallel over the batch dim, 65536 rows per core.
"""

import os
import numpy as np
from contextlib import ExitStack

ATOMS = 51
V_MAX = 10.0
V_MIN = -10.0
DELTA = (V_MAX - V_MIN) / (ATOMS - 1)
B = 524288
N_CORES = 8
ROWS = B // N_CORES          # 65536 rows per core
R = 64                       # rows packed per partition per megatile
MT = ROWS // (128 * R)       # 8 megatiles per core

_NC_CACHE = None
LAST_RESULT = None           # BassKernelResults of the most recent device run


def _build_nc():
    import concourse.bass as bass
    import concourse.tile as tile
    from concourse import bacc, mybir

    nc = bacc.Bacc(
        "TRN2",
        target_bir_lowering=False,
        debug=False,
        enable_asserts=True,
        num_devices=N_CORES,
        enable_partition_id=False,
    )
    f32 = mybir.dt.float32
    bf16 = mybir.dt.bfloat16
    u16 = mybir.dt.uint16

    # aws = anchor * (w * (1-2m)), bf16;  feat = feature, bf16
    aws = nc.dram_tensor("aws", [ROWS, ATOMS], bf16, kind="ExternalInput").ap()
    feat = nc.dram_tensor("feat", [ROWS, ATOMS], bf16, kind="ExternalInput").ap()
    acc = nc.dram_tensor("acc", [ATOMS, 2 * ATOMS], f32, kind="ExternalOutput").ap()

    # chunk schedule: big chunks while DMA-paced, small chunks at the end so
    # the post-DMA serial tail (ACT -> PE) drains fast
    chunks = [R] * (MT - 1) + [R // 4] * 4
    assert sum(chunks) * 128 == ROWS

    with ExitStack() as ctx:
        tc = ctx.enter_context(tile.TileContext(nc))
        singles = ctx.enter_context(tc.tile_pool(name="singles", bufs=1))
        loads = ctx.enter_context(tc.tile_pool(name="loads", bufs=8))
        mids = ctx.enter_context(tc.tile_pool(name="mids", bufs=3))
        psums = ctx.enter_context(tc.tile_pool(name="psums", bufs=1, space="PSUM"))

        eps_sb = singles.tile([128, 1], f32)
        nc.vector.memset(eps_sb, 1e-16)

        psum_acc = psums.tile([ATOMS, 2 * ATOMS], f32)

        row = 0
        for ci, rc in enumerate(chunks):
            f_v = feat[row * 128 : (row + rc) * 128, :].rearrange(
                "(p r) j -> p r j", r=rc
            )
            a_v = aws[row * 128 : (row + rc) * 128, :].rearrange(
                "(p r) j -> p r j", r=rc
            )
            row += rc

            # separate issue engines -> two independent DMA streams: the
            # feature stream (feeds the longer ACT->PE chain) is not queued
            # behind anchor transfers, and issue-side waits don't serialize
            f_t = loads.tile([128, rc, ATOMS], bf16, tag="f")
            nc.sync.dma_start(out=f_t, in_=f_v)
            # ab[:, 0] = |aws| (unsigned w*anchor), ab[:, 1] = signed aws (DMA)
            ab_t = loads.tile([128, 2, rc, ATOMS], bf16, tag="ab")
            nc.gpsimd.dma_start(out=ab_t[:, 1], in_=a_v)

            # logf = ln(feature + 1e-16), bf16
            logf_t = mids.tile([128, rc, ATOMS], bf16, tag="logf")
            nc.scalar.activation(
                logf_t.rearrange("p r j -> p (r j)"),
                f_t.rearrange("p r j -> p (r j)"),
                mybir.ActivationFunctionType.Ln,
                bias=eps_sb,
            )

            # abs via sign-bit mask (1-input DVE op -> 4x mode)
            nc.vector.tensor_scalar(
                ab_t[:, 0].rearrange("p r j -> p (r j)").bitcast(u16),
                ab_t[:, 1].rearrange("p r j -> p (r j)").bitcast(u16),
                0x7FFF,
                None,
                mybir.AluOpType.bitwise_and,
            )

            for r in range(rc):
                nc.tensor.matmul(
                    psum_acc,
                    lhsT=logf_t[:, r, :],
                    rhs=ab_t[:, :, r, :],
                    start=(ci == 0 and r == 0),
                    stop=(ci == len(chunks) - 1 and r == rc - 1),
                )

        out_sb = singles.tile([ATOMS, 2 * ATOMS], f32)
        nc.vector.tensor_copy(out_sb, psum_acc)
        nc.sync.dma_start(out=acc, in_=out_sb)

    nc.compile()
    return nc


def _get_nc():
    global _NC_CACHE
    if _NC_CACHE is None:
        _NC_CACHE = _build_nc()
    return _NC_CACHE


def _build_P(skew):
    """51x51 projection matrix for scalar skew, replicating reference f32 ops."""
    supports = np.linspace(V_MIN, V_MAX, ATOMS, dtype=np.float32)
    Tz = np.clip(np.float32(skew) + supports, np.float32(V_MIN), np.float32(V_MAX))
    b = (Tz - np.float32(V_MIN)) / np.float32(DELTA)
    l = np.floor(b).astype(np.int32)
    u = np.ceil(b).astype(np.int32)
    eq = l == u
    l = np.where((u > 0) & eq, l - 1, l)
    u = np.where((l < ATOMS - 1) & (l == u), u + 1, u)
    wl = u.astype(np.float32) - b
    wu = b - l.astype(np.float32)
    P = np.zeros((ATOMS, ATOMS), dtype=np.float64)
    np.add.at(P, (np.arange(ATOMS), l), wl.astype(np.float64))
    np.add.at(P, (np.arange(ATOMS), u), wu.astype(np.float64))
    return P


def run_device(in_maps, trace=False):
    """Run the SPMD bass kernel; returns list of per-core {'acc': [51,102]}."""
    global LAST_RESULT
    from concourse.bass_utils import run_bass_kernel_spmd

    LAST_RESULT = run_bass_kernel_spmd(
        _get_nc(), in_maps, core_ids=list(range(N_CORES)), trace=trace
    )
    return LAST_RESULT.results


def make_in_maps(anchor, feature, direction, weight):
    import ml_dtypes

    bf16 = ml_dtypes.bfloat16
    anchor = np.asarray(anchor, dtype=np.float32)
    feature = np.asarray(feature, dtype=np.float32)
    w = np.asarray(weight, dtype=np.float32)
    m = (np.asarray(direction) == 1)
    signed_w = np.where(m, -w, w).astype(np.float32)
    aws = np.ascontiguousarray((anchor * signed_w[:, None]).astype(bf16))
    feat = np.ascontiguousarray(feature.astype(bf16))

    in_maps = []
    for c in range(N_CORES):
        in_maps.append(
            {
                "aws": aws[c * ROWS : (c + 1) * ROWS],
                "feat": feat[c * ROWS : (c + 1) * ROWS],
            }
        )
    return in_maps


def reduce_host(results, skewness):
    P0 = _build_P(np.float32(skewness))                # direction == 0 -> +s
    P1 = _build_P(np.float32(-np.float32(skewness)))   # direction == 1 -> -s
    acc = np.zeros((ATOMS, 2 * ATOMS), dtype=np.float64)
    for r in results:
        acc += np.asarray(r["acc"], dtype=np.float64)
    Mw = acc[:, :ATOMS]       # [u, j]  sum_i w|a| logf
    Ms = acc[:, ATOMS:]       # [u, j]  sum_i w(1-2m) a logf
    M0 = (Mw + Ms) / 2        # direction == 0 rows
    M1 = (Mw - Ms) / 2        # direction == 1 rows
    contrib = np.sum(P0 * M0.T) + np.sum(P1 * M1.T)
    return np.asarray(np.float32(-contrib / B))


def kernel(anchor, feature, skewness, direction, weight):
    in_maps = make_in_maps(anchor, feature, direction, weight)
    results = run_device(in_maps, trace=bool(os.environ.get("KERNEL_TRACE")))
    return reduce_host(results, skewness)


# revision 31
# speedup vs baseline: 1.4549x; 1.4549x over previous
"""Trainium2 Bass kernel for the C51-style categorical projection loss.

Math
----
The reference computes, per batch row i (direction d_i in {0,1}, scalar
skewness s):

    skewed_anchor[i] = anchor[i] @ P_{d_i}          (row-local scatter-add)
    loss = -mean_i( w_i * (skewed_anchor[i] . log(feature[i] + 1e-16)) )

P_d is a 51x51 projection matrix depending only on the scalar skew
(+s for d=0, -s for d=1).  Rearranging the triple sum, the whole loss
reduces to two 51x51 batch-contraction matrices:

    Mw [u,j] = sum_i w_i            * logf[i,u] * anchor[i,j]
    Ms [u,j] = sum_i w_i (1 - 2m_i) * logf[i,u] * anchor[i,j]
    loss = -( sum P0 . ((Mw+Ms)/2)^T + sum P1 . ((Mw-Ms)/2)^T ) / B

The host pre-scales anchor by the signed weight w*(1-2m) and ships it (and
feature) as bf16 — half the HBM traffic of f32.  On device, the unsigned
operand is recovered with a bitwise-and (abs on the bf16 sign bit), and the
contraction over batch rows maps straight onto the TensorEngine (batch rows
on the 128 partitions).  No scatter at all on device; the 51x51 results and
the tiny P matrices are combined on the host.

Sharding: pure data parallel over the batch dim, 65536 rows per core.
"""

import os
import numpy as np
from contextlib import ExitStack

ATOMS = 51
APAD = 52                    # aws padded to 52 atoms (4-byte-aligned fp8 rows)
V_MAX = 10.0
V_MIN = -10.0
DELTA = (V_MAX - V_MIN) / (ATOMS - 1)
B = 524288
N_CORES = 8
ROWS = B // N_CORES          # 65536 rows per core
R = 64                       # rows packed per partition per megatile
MT = ROWS // (128 * R)       # 8 megatiles per core

_NC_CACHE = None
LAST_RESULT = None           # BassKernelResults of the most recent device run


def _build_nc():
    import concourse.bass as bass
    import concourse.tile as tile
    from concourse import bacc, mybir

    nc = bacc.Bacc(
        "TRN2",
        target_bir_lowering=False,
        debug=False,
        enable_asserts=True,
        num_devices=N_CORES,
        enable_partition_id=False,
    )
    f32 = mybir.dt.float32
    bf16 = mybir.dt.bfloat16
    fp8 = mybir.dt.float8e4
    u16 = mybir.dt.uint16

    # aws = anchor * (w * (1-2m)) in fp8e4m3 (padded to 52 cols); feat bf16
    aws = nc.dram_tensor("aws", [ROWS, APAD], fp8, kind="ExternalInput").ap()
    feat = nc.dram_tensor("feat", [ROWS, ATOMS], bf16, kind="ExternalInput").ap()
    acc = nc.dram_tensor("acc", [ATOMS, 2 * APAD], f32, kind="ExternalOutput").ap()

    # chunk schedule: big chunks while DMA-paced, small chunks at the end so
    # the post-DMA serial tail (ACT -> PE) drains fast
    chunks = [R] * (MT - 1) + [R // 4] * 4
    assert sum(chunks) * 128 == ROWS

    with ExitStack() as ctx:
        tc = ctx.enter_context(tile.TileContext(nc))
        singles = ctx.enter_context(tc.tile_pool(name="singles", bufs=1))
        loads = ctx.enter_context(tc.tile_pool(name="loads", bufs=8))
        mids = ctx.enter_context(tc.tile_pool(name="mids", bufs=3))
        psums = ctx.enter_context(tc.tile_pool(name="psums", bufs=1, space="PSUM"))

        eps_sb = singles.tile([128, 1], f32)
        nc.vector.memset(eps_sb, 1e-16)

        psum_acc = psums.tile([ATOMS, 2 * APAD], f32)

        row = 0
        for ci, rc in enumerate(chunks):
            f_v = feat[row * 128 : (row + rc) * 128, :].rearrange(
                "(p r) j -> p r j", r=rc
            )
            a_v = aws[row * 128 : (row + rc) * 128, :].rearrange(
                "(p r) j -> p r j", r=rc
            )
            row += rc

            # feature first: it feeds the longer (ACT -> PE) chain
            f_t = loads.tile([128, rc, ATOMS], bf16, tag="f")
            nc.sync.dma_start(out=f_t, in_=f_v)
            # ab[:, 0] = |aws| (unsigned w*anchor), ab[:, 1] = signed aws (DMA)
            ab_t = loads.tile([128, 2, rc, APAD], fp8, tag="ab")
            nc.sync.dma_start(out=ab_t[:, 1], in_=a_v)

            # logf = ln(feature + 1e-16), bf16
            logf_t = mids.tile([128, rc, ATOMS], bf16, tag="logf")
            nc.scalar.activation(
                logf_t.rearrange("p r j -> p (r j)"),
                f_t.rearrange("p r j -> p (r j)"),
                mybir.ActivationFunctionType.Ln,
                bias=eps_sb,
            )

            # abs of two packed fp8 per u16 lane via sign-bit mask
            # (1-input DVE op on 16-bit data -> 4x mode)
            nc.vector.tensor_scalar(
                ab_t[:, 0].rearrange("p r j -> p (r j)").bitcast(u16),
                ab_t[:, 1].rearrange("p r j -> p (r j)").bitcast(u16),
                0x7F7F,
                None,
                mybir.AluOpType.bitwise_and,
            )

            for r in range(rc):
                nc.tensor.matmul(
                    psum_acc,
                    lhsT=logf_t[:, r, :],
                    rhs=ab_t[:, :, r, :],
                    start=(ci == 0 and r == 0),
                    stop=(ci == len(chunks) - 1 and r == rc - 1),
                )

        out_sb = singles.tile([ATOMS, 2 * APAD], f32)
        nc.vector.tensor_copy(out_sb, psum_acc)
        nc.sync.dma_start(out=acc, in_=out_sb)

    nc.compile()
    return nc


def _get_nc():
    global _NC_CACHE
    if _NC_CACHE is None:
        _NC_CACHE = _build_nc()
    return _NC_CACHE


def _build_P(skew):
    """51x51 projection matrix for scalar skew, replicating reference f32 ops."""
    supports = np.linspace(V_MIN, V_MAX, ATOMS, dtype=np.float32)
    Tz = np.clip(np.float32(skew) + supports, np.float32(V_MIN), np.float32(V_MAX))
    b = (Tz - np.float32(V_MIN)) / np.float32(DELTA)
    l = np.floor(b).astype(np.int32)
    u = np.ceil(b).astype(np.int32)
    eq = l == u
    l = np.where((u > 0) & eq, l - 1, l)
    u = np.where((l < ATOMS - 1) & (l == u), u + 1, u)
    wl = u.astype(np.float32) - b
    wu = b - l.astype(np.float32)
    P = np.zeros((ATOMS, ATOMS), dtype=np.float64)
    np.add.at(P, (np.arange(ATOMS), l), wl.astype(np.float64))
    np.add.at(P, (np.arange(ATOMS), u), wu.astype(np.float64))
    return P


def run_device(in_maps, trace=False):
    """Run the SPMD bass kernel; returns list of per-core {'acc': [51,102]}."""
    global LAST_RESULT
    from concourse.bass_utils import run_bass_kernel_spmd

    LAST_RESULT = run_bass_kernel_spmd(
        _get_nc(), in_maps, core_ids=list(range(N_CORES)), trace=trace
    )
    return LAST_RESULT.results


def make_in_maps(anchor, feature, direction, weight):
    import ml_dtypes

    bf16 = ml_dtypes.bfloat16
    anchor = np.asarray(anchor, dtype=np.float32)
    feature = np.asarray(feature, dtype=np.float32)
    w = np.asarray(weight, dtype=np.float32)
    m = (np.asarray(direction) == 1)
    signed_w = np.where(m, -w, w).astype(np.float32)
    fp8 = ml_dtypes.float8_e4m3
    aws = np.zeros((B, APAD), dtype=fp8)
    aws[:, :ATOMS] = (anchor * signed_w[:, None]).astype(fp8)
    feat = np.ascontiguousarray(feature.astype(bf16))

    in_maps = []
    for c in range(N_CORES):
        in_maps.append(
            {
                "aws": aws[c * ROWS : (c + 1) * ROWS],
                "feat": feat[c * ROWS : (c + 1) * ROWS],
            }
        )
    return in_maps


def reduce_host(results, skewness):
    P0 = _build_P(np.float32(skewness))                # direction == 0 -> +s
    P1 = _build_P(np.float32(-np.float32(skewness)))   # direction == 1 -> -s
    acc = np.zeros((ATOMS, 2 * APAD), dtype=np.float64)
    for r in results:
        acc += np.asarray(r["acc"], dtype=np.float64)
    Mw = acc[:, :ATOMS]               # [u, j]  sum_i w|a| logf
    Ms = acc[:, APAD : APAD + ATOMS]  # [u, j]  sum_i w(1-2m) a logf
    M0 = (Mw + Ms) / 2        # direction == 0 rows
    M1 = (Mw - Ms) / 2        # direction == 1 rows
    contrib = np.sum(P0 * M0.T) + np.sum(P1 * M1.T)
    return np.asarray(np.float32(-contrib / B))


def kernel(anchor, feature, skewness, direction, weight):
    in_maps = make_in_maps(anchor, feature, direction, weight)
    results = run_device(in_maps, trace=bool(os.environ.get("KERNEL_TRACE")))
    return reduce_host(results, skewness)
